# revision 34
# baseline (speedup 1.0000x reference)
"""Trainium2 Bass kernel: CodeEncoder attention pooling, vocab-sharded
histogram form emitting per-core partial sums.

Math per bag: out = sum_c softmax(score(idx_c))_c * table[idx_c]. Scores
depend only on the vocab id (score = W2 tanh(W1 e + b1); b2 cancels in
softmax), so with per-bag vocab counts Cnt[v, bag] (host-built):

    g(v) = exp(score_v)                    (device, score-table MLP)
    num  = (g*table)^T @ Cnt  [bags, 257]  (dense matmul, ones col -> Z)
    out  = num / Z

Sharding: VOCAB-sharded. Core k owns vocab slice [2560k, 2560k+2560):
it runs the score MLP on its slice only (1/8 the table traffic and MLP
flops of the batch-parallel form) and accumulates bf16 partial (num|Z)
rows for ALL 3200 bags over its slice, K-contiguous per 128-bag tile
so the PE never idles. The unshard step sums the 8 partial outputs and
normalizes. (An on-device ReduceScatter was measured at ~65us fixed
subsystem warmup + ~30us/MB + ~12us trigger latency in this runtime —
see kernel_rs.py for that variant; it is 60-70us slower end-to-end.)

Length-0 bags (softmax fully masked -> uniform mean of all 64 codes)
ride the same matmul stream as 256 extra "slot" columns (2 leading
tiles) whose counts are the full-64-code histogram and whose rhs is
the RAW (unscaled) table with its ones column, so their partial rows
normalize to the plain mean. The host maps slots onto their bags.

cnt tiles alternate between the sync and gpsimd DMA queues: one
queue-ring sustains only ~115 GB/s here, short of the PE's ~150 GB/s
cnt consumption; two rings together stay ahead.
"""

import sys

if "/opt/trn_rl_repo" not in sys.path:
    sys.path.insert(0, "/opt/trn_rl_repo")

from contextlib import ExitStack

import numpy as np

B, V, C = 64, 50, 64
NUM_CODE, D, H = 20000, 256, 128
NCORES = 8
BPC = B // NCORES          # batches per core
BAGS = BPC * V             # 400 bags owned per core
GBAGS = B * V              # 3200 global bags
VP = 20480                 # padded vocab
VSL = VP // NCORES         # 2560 vocab per core
NCH = VSL // 128           # 20 vocab chunks per core
NSL = 512                  # score-MLP slice (one f32 psum bank)
NW = D + 2                 # rhs width: 256 emb + ones col + pad
NSLOT = 128                # global len-0 slot columns (one tile)
NST = NSLOT // 128         # 1 slot tile (computed first)
NT = GBAGS // 128 + NST    # 26 matmul tiles
NCOL = NSLOT + GBAGS       # 3328 partial rows

_cache = {}


def _build_program():
    import concourse.bass as bass  # noqa: F401
    import concourse.tile as tile
    from concourse import bacc, mybir

    f16 = mybir.dt.float16
    f32 = mybir.dt.float32
    bf16 = mybir.dt.bfloat16
    f8 = mybir.dt.float8e4

    nc = bacc.Bacc("TRN2", target_bir_lowering=False, debug=False,
                   num_devices=NCORES)

    tabt_d = nc.dram_tensor("tabt", [128, 2 * VSL], f8, kind="ExternalInput")
    rhsc_d = nc.dram_tensor("rhsc", [128, NCH * NW], f16, kind="ExternalInput")
    cnt_d = nc.dram_tensor("cnt", [128, NT * NCH * 128], f8,
                           kind="ExternalInput")
    w1t_d = nc.dram_tensor("w1t", [D, H], f16, kind="ExternalInput")
    w2c_d = nc.dram_tensor("w2c", [H, 1], f16, kind="ExternalInput")
    b1_d = nc.dram_tensor("b1", [H, 1], f32, kind="ExternalInput")
    part_d = nc.dram_tensor("part", [NCOL, NW], bf16, kind="ExternalOutput")

    with tile.TileContext(nc) as tc, ExitStack() as ctx:
        const = ctx.enter_context(tc.tile_pool(name="const", bufs=1))
        cntp = ctx.enter_context(tc.tile_pool(name="cntp", bufs=8))
        hp = ctx.enter_context(tc.tile_pool(name="hp", bufs=2))
        obp = ctx.enter_context(tc.tile_pool(name="obp", bufs=6))
        php = ctx.enter_context(tc.tile_pool(name="ph", bufs=2, space="PSUM"))
        gpp = ctx.enter_context(tc.tile_pool(name="gp", bufs=1, space="PSUM"))
        psp = ctx.enter_context(tc.tile_pool(name="ps", bufs=2, space="PSUM"))

        # --- constants; tabt pieces head the sync queue (earliest issuer;
        # first ~8us is fixed engine-barrier preamble), weights + rhsc on
        # gpsimd so the MLP can start as soon as piece 0 lands ---
        w1t_sb = const.tile([128, 2, H], f16)
        nc.gpsimd.dma_start(w1t_sb[:, 0, :], w1t_d.ap()[0:128, :])
        nc.gpsimd.dma_start(w1t_sb[:, 1, :], w1t_d.ap()[128:256, :])
        w2c_sb = const.tile([H, 1], f16)
        nc.gpsimd.dma_start(w2c_sb[:], w2c_d.ap())
        b1_sb = const.tile([H, 1], f32)
        nc.gpsimd.dma_start(b1_sb[:], b1_d.ap())
        tabt_sb = const.tile([128, 2, VSL], f8)
        rhsc_sb = const.tile([128, NCH, NW], f16)
        # tabt_d is half-major [128, 2, VSL]: two contiguous DMAs
        for h in range(2):
            nc.sync.dma_start(tabt_sb[:, h, :],
                              tabt_d.ap()[:, h * VSL:(h + 1) * VSL])
        CPS = NCH // (VSL // NSL)  # rhs chunks per MLP slice
        for s in range(VSL // NSL):
            nc.gpsimd.dma_start(
                rhsc_sb[:, s * CPS:(s + 1) * CPS, :].rearrange(
                    "p a b -> p (a b)"),
                rhsc_d.ap()[:, s * CPS * NW:(s + 1) * CPS * NW])

        g_sb = const.tile([128, NCH], f32)
        tg_sb = const.tile([128, NCH, NW], f16)
        g_ps = gpp.tile([128, NCH], f32)

        # --- score MLP over the vocab slice, then per-chunk rhs scaling ---
        for s in range(VSL // NSL):
            ssl = slice(s * NSL, (s + 1) * NSL)
            ph = php.tile([128, NSL], f32)
            nc.tensor.matmul(ph[:], w1t_sb[:, 0, :], tabt_sb[:, 0, ssl],
                             start=True, stop=False)
            nc.tensor.matmul(ph[:], w1t_sb[:, 1, :], tabt_sb[:, 1, ssl],
                             start=False, stop=True)
            h1 = hp.tile([128, NSL], f16)
            nc.scalar.activation(h1[:], ph[:],
                                 mybir.ActivationFunctionType.Tanh,
                                 bias=b1_sb[:], scale=1.0)
            for k in range(NSL // 128):
                j = s * (NSL // 128) + k
                nc.tensor.matmul(g_ps[:, j:j + 1],
                                 h1[:, k * 128:(k + 1) * 128], w2c_sb[:],
                                 start=True, stop=True)
            jsl = slice(s * (NSL // 128), (s + 1) * (NSL // 128))
            nc.scalar.activation(g_sb[:, jsl], g_ps[:, jsl],
                                 mybir.ActivationFunctionType.Exp)
            for k in range(NSL // 128):
                j = s * (NSL // 128) + k
                nc.vector.tensor_scalar(tg_sb[:, j, :], rhsc_sb[:, j, :],
                                        g_sb[:, j:j + 1], None,
                                        mybir.AluOpType.mult)

        # --- main loop: slot tiles (raw rhs) first, then 25 bag tiles;
        # K-contiguous per tile; cnt stream alternates DMA queues ---
        for t in range(NT):
            ct = cntp.tile([128, NCH, 128], f8)
            q = nc.sync if t % 2 == 0 else nc.gpsimd
            q.dma_start(ct[:].rearrange("p a b -> p (a b)"),
                        cnt_d.ap()[:, t * NCH * 128:(t + 1) * NCH * 128])
            ps = psp.tile([128, NW], f32)
            src = rhsc_sb if t < NST else tg_sb
            for j in range(NCH):
                nc.tensor.matmul(ps[:], ct[:, j, :], src[:, j, :],
                                 start=(j == 0), stop=(j == NCH - 1))
            ob = obp.tile([128, NW], bf16)
            nc.vector.tensor_copy(ob[:], ps[:])
            # last stores ride sync: HWDGE drains in ns, the SWDGE ring
            # drain costs ~3.8us when it holds the final packet
            q2 = nc.gpsimd if (t % 2 == 0 and t < NT - 2) else nc.sync
            q2.dma_start(part_d.ap()[t * 128:(t + 1) * 128, :], ob[:])

    nc.compile()
    return nc


def _prep_shared(embed_table, W1, b1, W2):
    """Per-core-sliceable views of the table + tiny MLP weights."""
    t16 = embed_table.astype(np.float16)                      # [20000, 256]
    tabt = np.zeros((D, VP), np.float16)
    tabt[:, :NUM_CODE] = t16.T
    rhsc = np.zeros((VP, NW), np.float16)
    rhsc[:NUM_CODE, :D] = t16
    rhsc[:NUM_CODE, D] = 1.0
    w1t = np.ascontiguousarray(W1.astype(np.float16).T)       # [256, 128]
    w2c = np.ascontiguousarray(W2.astype(np.float16).reshape(H, 1))
    b1c = np.ascontiguousarray(b1.astype(np.float32).reshape(H, 1))
    return dict(tabt=tabt, rhsc=rhsc, w1t=w1t, w2c=w2c, b1=b1c)


def build_in_maps(input_code, length_code, shared):
    import ml_dtypes

    codes = input_code.reshape(GBAGS, C).astype(np.int64)
    lens = length_code.reshape(GBAGS).astype(np.int64)

    # column order: slot tiles first (8 cores x 32), then bags core-major
    cnt = np.zeros((VP, NCOL), np.float32)
    valid = np.arange(C)[None, :] < lens[:, None]
    bb, cc = np.nonzero(valid)
    np.add.at(cnt, (codes[bb, cc], NSLOT + bb), 1.0)

    len0_global = np.nonzero(lens == 0)[0][:NSLOT]
    for s, b in enumerate(len0_global):
        np.add.at(cnt, (codes[b], s), 1.0)

    in_maps = []
    for core in range(NCORES):
        vs = slice(core * VSL, (core + 1) * VSL)
        cntl = np.ascontiguousarray(
            cnt[vs].reshape(NCH, 128, NT, 128).transpose(1, 2, 0, 3)
        ).astype(ml_dtypes.float8_e4m3).reshape(128, NT * NCH * 128)
        tabtc = np.ascontiguousarray(
            shared["tabt"][:, vs].reshape(2, 128, VSL).transpose(1, 0, 2)
        ).astype(ml_dtypes.float8_e4m3).reshape(128, 2 * VSL)
        rhscc = np.ascontiguousarray(
            shared["rhsc"][vs].reshape(NCH, 128, NW).transpose(1, 0, 2)
        ).reshape(128, NCH * NW)
        in_maps.append(dict(tabt=tabtc, rhsc=rhscc, cnt=cntl,
                            w1t=shared["w1t"], w2c=shared["w2c"],
                            b1=shared["b1"]))
    return in_maps, len0_global


def kernel(input_code, length_code, embed_table, W1, b1, W2, b2):
    from concourse.bass_utils import run_bass_kernel_spmd

    if "nc" not in _cache:
        _cache["nc"] = _build_program()
    nc = _cache["nc"]

    shared = _prep_shared(np.asarray(embed_table), np.asarray(W1),
                          np.asarray(b1), np.asarray(W2))
    in_maps, len0_global = build_in_maps(np.asarray(input_code),
                                         np.asarray(length_code), shared)
    res = run_bass_kernel_spmd(nc, in_maps, core_ids=list(range(NCORES)))
    # unshard: vocab-sharded partial (num|Z) rows sum across cores, then
    # normalize; len-0 bags take their slot rows (uniform mean, Z = 64)
    acc = np.zeros((NCOL, NW), np.float32)
    for c in range(NCORES):
        acc += res.results[c]["part"].astype(np.float32)
    z = acc[:, D:D + 1]
    full = acc[:, :D] / np.where(z == 0.0, 1.0, z)
    out = full[NSLOT:].copy()
    for s, b in enumerate(len0_global):
        out[b] = full[s]
    return out.reshape(B, V, D).astype(np.float32)


# revision 35
# speedup vs baseline: 1.0358x; 1.0358x over previous
"""Trainium2 Bass kernel: CodeEncoder attention pooling, vocab-sharded
histogram form emitting per-core partial sums.

Math per bag: out = sum_c softmax(score(idx_c))_c * table[idx_c]. Scores
depend only on the vocab id (score = W2 tanh(W1 e + b1); b2 cancels in
softmax), so with per-bag vocab counts Cnt[v, bag] (host-built):

    g(v) = exp(score_v)                    (device, score-table MLP)
    num  = (g*table)^T @ Cnt  [bags, 257]  (dense matmul, ones col -> Z)
    out  = num / Z

Sharding: VOCAB-sharded. Core k owns vocab slice [2560k, 2560k+2560):
it runs the score MLP on its slice only (1/8 the table traffic and MLP
flops of the batch-parallel form) and accumulates bf16 partial (num|Z)
rows for ALL 3200 bags over its slice, K-contiguous per 128-bag tile
so the PE never idles. The unshard step sums the 8 partial outputs and
normalizes. (An on-device ReduceScatter was measured at ~65us fixed
subsystem warmup + ~30us/MB + ~12us trigger latency in this runtime —
see kernel_rs.py for that variant; it is 60-70us slower end-to-end.)

Length-0 bags (softmax fully masked -> uniform mean of all 64 codes)
ride the same matmul stream as 256 extra "slot" columns (2 leading
tiles) whose counts are the full-64-code histogram and whose rhs is
the RAW (unscaled) table with its ones column, so their partial rows
normalize to the plain mean. The host maps slots onto their bags.

cnt tiles alternate between the sync and gpsimd DMA queues: one
queue-ring sustains only ~115 GB/s here, short of the PE's ~150 GB/s
cnt consumption; two rings together stay ahead.
"""

import sys

if "/opt/trn_rl_repo" not in sys.path:
    sys.path.insert(0, "/opt/trn_rl_repo")

from contextlib import ExitStack

import numpy as np

B, V, C = 64, 50, 64
NUM_CODE, D, H = 20000, 256, 128
NCORES = 8
BPC = B // NCORES          # batches per core
BAGS = BPC * V             # 400 bags owned per core
GBAGS = B * V              # 3200 global bags
VP = 20480                 # padded vocab
VSL = VP // NCORES         # 2560 vocab per core
NCH = VSL // 128           # 20 vocab chunks per core
NSL = 512                  # score-MLP slice (one f32 psum bank)
NW = D + 2                 # rhs width: 256 emb + ones col + pad
NSLOT = 128                # global len-0 slot columns (one tile)
NST = NSLOT // 128         # 1 slot tile (computed first)
NT = GBAGS // 128 + NST    # 26 matmul tiles
NCOL = NSLOT + GBAGS       # 3328 partial rows

_cache = {}


def _build_program():
    import concourse.bass as bass  # noqa: F401
    import concourse.tile as tile
    from concourse import bacc, mybir

    f16 = mybir.dt.float16
    f32 = mybir.dt.float32
    bf16 = mybir.dt.bfloat16
    f8 = mybir.dt.float8e4

    nc = bacc.Bacc("TRN2", target_bir_lowering=False, debug=False,
                   num_devices=NCORES)

    tabt_d = nc.dram_tensor("tabt", [128, 2 * VSL], f8, kind="ExternalInput")
    rhsc_d = nc.dram_tensor("rhsc", [128, NCH * NW], f16, kind="ExternalInput")
    cnt_d = nc.dram_tensor("cnt", [128, NT * NCH * 128], f8,
                           kind="ExternalInput")
    w1t_d = nc.dram_tensor("w1t", [D, H], f16, kind="ExternalInput")
    w2c_d = nc.dram_tensor("w2c", [H, 1], f16, kind="ExternalInput")
    b1_d = nc.dram_tensor("b1", [H, 1], f32, kind="ExternalInput")
    part_d = nc.dram_tensor("part", [NCOL, NW], bf16, kind="ExternalOutput")

    with tile.TileContext(nc) as tc, ExitStack() as ctx:
        const = ctx.enter_context(tc.tile_pool(name="const", bufs=1))
        cntp = ctx.enter_context(tc.tile_pool(name="cntp", bufs=8))
        hp = ctx.enter_context(tc.tile_pool(name="hp", bufs=2))
        obp = ctx.enter_context(tc.tile_pool(name="obp", bufs=6))
        php = ctx.enter_context(tc.tile_pool(name="ph", bufs=2, space="PSUM"))
        gpp = ctx.enter_context(tc.tile_pool(name="gp", bufs=1, space="PSUM"))
        psp = ctx.enter_context(tc.tile_pool(name="ps", bufs=2, space="PSUM"))

        # --- constants; tabt pieces head the sync queue (earliest issuer;
        # first ~8us is fixed engine-barrier preamble), weights + rhsc on
        # gpsimd so the MLP can start as soon as piece 0 lands ---
        w1t_sb = const.tile([128, 2, H], f16)
        nc.gpsimd.dma_start(w1t_sb[:, 0, :], w1t_d.ap()[0:128, :])
        nc.gpsimd.dma_start(w1t_sb[:, 1, :], w1t_d.ap()[128:256, :])
        w2c_sb = const.tile([H, 1], f16)
        nc.gpsimd.dma_start(w2c_sb[:], w2c_d.ap())
        b1_sb = const.tile([H, 1], f32)
        nc.gpsimd.dma_start(b1_sb[:], b1_d.ap())
        tabt_sb = const.tile([128, 2, VSL], f8)
        rhsc_sb = const.tile([128, NCH, NW], f16)
        # tabt_d is piece-contiguous: [128, 5 pieces, 2 halves, 512]
        for s in range(VSL // NSL):
            ssl = slice(s * NSL, (s + 1) * NSL)
            nc.sync.dma_start(
                tabt_sb[:, :, ssl],
                tabt_d.ap()[:, :].rearrange(
                    "p (s a b) -> p s (a b)", s=VSL // NSL, a=2)[:, s, :])
        CPS = NCH // (VSL // NSL)  # rhs chunks per MLP slice
        for s in range(VSL // NSL):
            nc.gpsimd.dma_start(
                rhsc_sb[:, s * CPS:(s + 1) * CPS, :].rearrange(
                    "p a b -> p (a b)"),
                rhsc_d.ap()[:, s * CPS * NW:(s + 1) * CPS * NW])

        g_sb = const.tile([128, NCH], f32)
        tg_sb = const.tile([128, NCH, NW], f16)
        g_ps = gpp.tile([128, NCH], f32)

        # --- score MLP over the vocab slice, then per-chunk rhs scaling ---
        for s in range(VSL // NSL):
            ssl = slice(s * NSL, (s + 1) * NSL)
            ph = php.tile([128, NSL], f32)
            nc.tensor.matmul(ph[:], w1t_sb[:, 0, :], tabt_sb[:, 0, ssl],
                             start=True, stop=False)
            nc.tensor.matmul(ph[:], w1t_sb[:, 1, :], tabt_sb[:, 1, ssl],
                             start=False, stop=True)
            h1 = hp.tile([128, NSL], f16)
            nc.scalar.activation(h1[:], ph[:],
                                 mybir.ActivationFunctionType.Tanh,
                                 bias=b1_sb[:], scale=1.0)
            for k in range(NSL // 128):
                j = s * (NSL // 128) + k
                nc.tensor.matmul(g_ps[:, j:j + 1],
                                 h1[:, k * 128:(k + 1) * 128], w2c_sb[:],
                                 start=True, stop=True)
            jsl = slice(s * (NSL // 128), (s + 1) * (NSL // 128))
            nc.scalar.activation(g_sb[:, jsl], g_ps[:, jsl],
                                 mybir.ActivationFunctionType.Exp)
            for k in range(NSL // 128):
                j = s * (NSL // 128) + k
                nc.vector.tensor_scalar(tg_sb[:, j, :], rhsc_sb[:, j, :],
                                        g_sb[:, j:j + 1], None,
                                        mybir.AluOpType.mult)

        # --- main loop: slot tiles (raw rhs) first, then 25 bag tiles;
        # K-contiguous per tile; cnt stream alternates DMA queues ---
        for t in range(NT):
            ct = cntp.tile([128, NCH, 128], f8)
            q = nc.sync if t % 2 == 0 else nc.gpsimd
            q.dma_start(ct[:].rearrange("p a b -> p (a b)"),
                        cnt_d.ap()[:, t * NCH * 128:(t + 1) * NCH * 128])
            ps = psp.tile([128, NW], f32)
            src = rhsc_sb if t < NST else tg_sb
            for j in range(NCH):
                nc.tensor.matmul(ps[:], ct[:, j, :], src[:, j, :],
                                 start=(j == 0), stop=(j == NCH - 1))
            ob = obp.tile([128, NW], bf16)
            nc.vector.tensor_copy(ob[:], ps[:])
            # last stores ride sync: HWDGE drains in ns, the SWDGE ring
            # drain costs ~3.8us when it holds the final packet
            q2 = nc.gpsimd if (t % 2 == 0 and t < NT - 2) else nc.sync
            q2.dma_start(part_d.ap()[t * 128:(t + 1) * 128, :], ob[:])

    nc.compile()
    return nc


def _prep_shared(embed_table, W1, b1, W2):
    """Per-core-sliceable views of the table + tiny MLP weights."""
    t16 = embed_table.astype(np.float16)                      # [20000, 256]
    tabt = np.zeros((D, VP), np.float16)
    tabt[:, :NUM_CODE] = t16.T
    rhsc = np.zeros((VP, NW), np.float16)
    rhsc[:NUM_CODE, :D] = t16
    rhsc[:NUM_CODE, D] = 1.0
    w1t = np.ascontiguousarray(W1.astype(np.float16).T)       # [256, 128]
    w2c = np.ascontiguousarray(W2.astype(np.float16).reshape(H, 1))
    b1c = np.ascontiguousarray(b1.astype(np.float32).reshape(H, 1))
    return dict(tabt=tabt, rhsc=rhsc, w1t=w1t, w2c=w2c, b1=b1c)


def build_in_maps(input_code, length_code, shared):
    import ml_dtypes

    codes = input_code.reshape(GBAGS, C).astype(np.int64)
    lens = length_code.reshape(GBAGS).astype(np.int64)

    # column order: slot tiles first (8 cores x 32), then bags core-major
    cnt = np.zeros((VP, NCOL), np.float32)
    valid = np.arange(C)[None, :] < lens[:, None]
    bb, cc = np.nonzero(valid)
    np.add.at(cnt, (codes[bb, cc], NSLOT + bb), 1.0)

    len0_global = np.nonzero(lens == 0)[0][:NSLOT]
    for s, b in enumerate(len0_global):
        np.add.at(cnt, (codes[b], s), 1.0)

    in_maps = []
    for core in range(NCORES):
        vs = slice(core * VSL, (core + 1) * VSL)
        cntl = np.ascontiguousarray(
            cnt[vs].reshape(NCH, 128, NT, 128).transpose(1, 2, 0, 3)
        ).astype(ml_dtypes.float8_e4m3).reshape(128, NT * NCH * 128)
        tabtc = np.ascontiguousarray(
            shared["tabt"][:, vs].reshape(2, 128, VSL // NSL, NSL)
            .transpose(1, 2, 0, 3)
        ).astype(ml_dtypes.float8_e4m3).reshape(128, 2 * VSL)
        rhscc = np.ascontiguousarray(
            shared["rhsc"][vs].reshape(NCH, 128, NW).transpose(1, 0, 2)
        ).reshape(128, NCH * NW)
        in_maps.append(dict(tabt=tabtc, rhsc=rhscc, cnt=cntl,
                            w1t=shared["w1t"], w2c=shared["w2c"],
                            b1=shared["b1"]))
    return in_maps, len0_global


def kernel(input_code, length_code, embed_table, W1, b1, W2, b2):
    from concourse.bass_utils import run_bass_kernel_spmd

    if "nc" not in _cache:
        _cache["nc"] = _build_program()
    nc = _cache["nc"]

    shared = _prep_shared(np.asarray(embed_table), np.asarray(W1),
                          np.asarray(b1), np.asarray(W2))
    in_maps, len0_global = build_in_maps(np.asarray(input_code),
                                         np.asarray(length_code), shared)
    res = run_bass_kernel_spmd(nc, in_maps, core_ids=list(range(NCORES)))
    # unshard: vocab-sharded partial (num|Z) rows sum across cores, then
    # normalize; len-0 bags take their slot rows (uniform mean, Z = 64)
    acc = np.zeros((NCOL, NW), np.float32)
    for c in range(NCORES):
        acc += res.results[c]["part"].astype(np.float32)
    z = acc[:, D:D + 1]
    full = acc[:, :D] / np.where(z == 0.0, 1.0, z)
    out = full[NSLOT:].copy()
    for s, b in enumerate(len0_global):
        out[b] = full[s]
    return out.reshape(B, V, D).astype(np.float32)
